# revision 38
# baseline (speedup 1.0000x reference)
"""AttnBlock (GroupNorm + 4096-token single-head attention + residual) on 8 trn2 cores.

Sharding: 2 cores per batch sample. Each core computes GroupNorm + K/V for the
full sample (duplicated within the pair) and attention for half the queries
(2048 of 4096). A single SPMD program serves both halves: the host rotates the
sample's spatial columns so each core's query half always sits at columns
0..2047 (attention is permutation-invariant over keys; GroupNorm stats are
permutation-invariant over spatial positions).

All heavy matmuls run in fp8(e4m3) with perf_mode=DoubleRow (K=256 per pass,
2x PE throughput vs bf16; the DR matmul rate is moving-operand-stream-bound at
~216ns per [128,2,512]x[128,2,512] pass, LDWEIGHTS fully hidden). The softmax
exp is computed as exp(s*scale - 2.5) so the fp8 range (TRN e4m3 max = 240) is
never exceeded; the PV accumulators are scaled by 1/16 on both the numerator
(ao eviction scale) and denominator (ones-matrix value) so the fp8 ao tiles
stay in range and the factors cancel exactly at the final division. The
residual path stays exact fp32 (separate DMA of the fp32 input columns).

GroupNorm: each 16-channel group lives inside one 128-channel partition tile,
so stats are per-tile independent. x arrives pre-cast to fp8 (2x DVE/ACT stats
rate, half the DMA bytes; quantization noise is far below the fp8 h noise).
Per tile: DVE sum + ACT square-accumulate, then one tiny PE matmul against a
0/1 group-indicator (junk DoubleRow matmuls on the landed tile keep the PE HAM
warm through the stats pipeline). The scalar (mean,var->scale,bias) chain is
batched across all four tiles into a single DVE/ACT pass so its cross-engine
latency is paid once, then one broadcast matmul restores per-channel columns.
x owns the sync DMA queue; weights/consts ride the gpsimd (SWDGE) queue so
their descriptor pushes don't delay the ACT stats stream. Hard-won scheduling
rules, all measured: do NOT split x across two DMA queues or use half-tile
descriptors (globally slows every engine ~1.2x on this part); do NOT put
mid-chunk epilogue work on the ACT queue (FIFO behind the next chunk's exp
stream); do NOT use GpSimd tensor ops for the epilogue (slow).
"""

import sys

for _p in ("/opt/trn_rl_repo", "/root/.axon_site/_ro/trn_rl_repo"):
    if _p not in sys.path:
        sys.path.append(_p)

import ml_dtypes
import numpy as np

C = 512
N = 4096
NQ = 2048
P = 128
CT = C // P  # 4 c-tiles
NKB = N // P  # 32 nk blocks
QCH = NQ // 512  # 4 q chunks of 512
EPS = 1e-5
SCALE = float(C) ** -0.5
EXP_BIAS = -2.5  # exp(s*SCALE - 2.5): keeps fp8 et <= ~25; cancels in division
AO_SCALE = 1.0 / 16.0  # unnormalized PV sums reach ~550; scale into fp8 range

_cache = {}


def _build():
    import concourse.bacc as bacc
    import concourse.bass as bass
    import concourse.mybir as mybir
    import concourse.tile as tile

    f32 = mybir.dt.float32
    fp8 = mybir.dt.float8e4
    bf16 = mybir.dt.bfloat16
    AF = mybir.ActivationFunctionType
    ALU = mybir.AluOpType
    AX = mybir.AxisListType
    DR = mybir.MatmulPerfMode.DoubleRow

    nc = bacc.Bacc("TRN2", target_bir_lowering=False, debug=False, num_devices=8)

    xb_d = nc.dram_tensor("xb", [C, N], fp8, kind="ExternalInput")
    xr_d = nc.dram_tensor("xr", [C, NQ], f32, kind="ExternalInput")
    w3_d = {
        nm: nc.dram_tensor(nm, [P, CT, C], fp8, kind="ExternalInput")
        for nm in ("wq3", "wk3", "wv3", "wo3")
    }
    col_d = {
        nm: nc.dram_tensor(nm, [P, CT], f32, kind="ExternalInput")
        for nm in ("bqc", "bkc", "boc", "gnwc", "gnbc")
    }
    bvb_d = nc.dram_tensor("bvb", [P, C], f32, kind="ExternalInput")
    g_d = nc.dram_tensor("gmat", [P, 8], f32, kind="ExternalInput")
    gt_d = nc.dram_tensor("gtmat", [8, P], f32, kind="ExternalInput")
    out_d = nc.dram_tensor("out", [C, NQ], f32, kind="ExternalOutput")

    xb_t = xb_d.ap().rearrange("(t p) n -> t p n", p=P)
    xr_t = xr_d.ap().rearrange("(t p) n -> t p n", p=P)
    out_t = out_d.ap().rearrange("(t p) n -> t p n", p=P)

    with tile.TileContext(nc) as tc:
        with (
            tc.tile_pool(name="const", bufs=1) as const,
            tc.tile_pool(name="work", bufs=3) as work,
            tc.tile_pool(name="wtp", bufs=1) as wtp,
            tc.tile_pool(name="hp", bufs=1) as hp,
            tc.tile_pool(name="ps_s", bufs=2, space="PSUM") as ps_s,
            tc.tile_pool(name="ps_o", bufs=4, space="PSUM") as ps_o,
            tc.tile_pool(name="ps_d", bufs=2, space="PSUM") as ps_d,
        ):
            # ---- constants (vector DMA queue; x owns sync, epilogue reuses it) ----
            ones3 = const.tile([P, 2, P], fp8)
            nc.vector.memset(ones3, AO_SCALE)
            eps8 = const.tile([8, 1], f32)
            nc.vector.memset(eps8, EPS)
            ebias = const.tile([P, 1], f32)
            nc.vector.memset(ebias, EXP_BIAS)
            gmat = const.tile([P, 8], f32)
            nc.gpsimd.dma_start(gmat, g_d.ap())
            gtmat = const.tile([8, P], f32)
            nc.gpsimd.dma_start(gtmat, gt_d.ap())

            cols = {}
            for nm in ("bqc", "bkc", "boc", "gnwc", "gnbc"):
                cols[nm] = const.tile([P, CT], f32, tag=f"c_{nm}", name=f"c_{nm}")
                nc.gpsimd.dma_start(cols[nm], col_d[nm].ap())
            bv_bcast = const.tile([P, C], f32)
            nc.gpsimd.dma_start(bv_bcast, bvb_d.ap())

            w3 = {}
            for nm in ("wk3", "wq3", "wv3", "wo3"):
                w3[nm] = wtp.tile([P, CT, C], fp8, tag=nm, name=nm)
                nc.gpsimd.dma_start(w3[nm], w3_d[nm].ap())

            h_all = hp.tile([P, CT, N], fp8)

            with tc.tile_pool(name="xp", bufs=1) as xp:
                xts = []
                psg_all = ps_d.tile([8, 2 * CT], f32, tag="d", name="psg_all")
                for t in range(CT):
                    xt = xp.tile([P, N], fp8, tag=f"x{t}", name=f"x{t}")
                    nc.sync.dma_start(xt, xb_t[t])
                    xts.append(xt)

                    # per-tile raw stats: DVE sum + ACT square-accumulate
                    # (free-axis reductions only exist on DVE/ACT; both are
                    # saturated here, so this pipeline is the preamble floor)
                    st2 = work.tile([P, 2], f32, tag="st2", bufs=2)
                    nc.vector.tensor_reduce(
                        out=st2[:, 0:1], in_=xt, axis=AX.X, op=ALU.add
                    )
                    junk = xp.tile([P, N], bf16, tag="junk", bufs=2)
                    nc.scalar.activation(
                        out=junk, in_=xt, func=AF.Square, accum_out=st2[:, 1:2]
                    )
                    nc.tensor.matmul(
                        psg_all[:, 2 * t : 2 * t + 2],
                        lhsT=gmat,
                        rhs=st2,
                        start=True,
                        stop=True,
                    )
                    # HAM warm-up: junk matmuls on the already-landed x tile
                    # fill the PE gaps of the stats pipeline so the k-phase
                    # starts at 2.4GHz instead of cold 1.2GHz
                    for _w in range(8):
                        jps = ps_s.tile([P, 512], f32, tag="s", name="jps")
                        nc.tensor.matmul(
                            jps,
                            lhsT=ones3,
                            rhs=xt[:, :1024].rearrange("p (k n) -> p k n", k=2),
                            start=True,
                            stop=True,
                            perf_mode=DR,
                        )

                # ---- batched (mean,var)->(scale,bias) chain for all tiles ----
                mq8 = work.tile([8, 4, 2], f32)  # [g, t, (mean, E[x^2])]
                nc.scalar.mul(
                    mq8, psg_all.rearrange("g (t c) -> g t c", c=2), 1.0 / (16.0 * 4096.0)
                )
                var4 = work.tile([8, 4], f32)
                nc.vector.tensor_mul(var4, mq8[:, :, 0], mq8[:, :, 0])
                nc.vector.tensor_sub(var4, mq8[:, :, 1], var4)
                nc.scalar.activation(out=var4, in_=var4, func=AF.Sqrt, bias=eps8)
                rstd4 = work.tile([8, 4], f32)
                nc.vector.reciprocal(rstd4, var4)
                mr8 = work.tile([8, 4, 2], f32)  # [g, t, (mean, rstd)]
                nc.vector.tensor_copy(mr8[:, :, 0], mq8[:, :, 0])
                nc.vector.tensor_copy(mr8[:, :, 1], rstd4)
                psc8 = ps_d.tile([P, 8], f32, tag="d", name="psc8")
                nc.tensor.matmul(psc8, lhsT=gtmat, rhs=mr8, start=True, stop=True)
                # per-channel scale/bias columns + fp8 h apply (DVE/ACT split)
                for t in range(CT):
                    scale_c = work.tile([P, 1], f32, tag="scale_c", bufs=2)
                    nc.vector.tensor_mul(
                        scale_c, psc8[:, 2 * t + 1 : 2 * t + 2], cols["gnwc"][:, t : t + 1]
                    )
                    bias_c = work.tile([P, 1], f32, tag="bias_c", bufs=2)
                    nc.vector.tensor_mul(bias_c, psc8[:, 2 * t : 2 * t + 1], scale_c)
                    nc.vector.tensor_sub(bias_c, cols["gnbc"][:, t : t + 1], bias_c)
                    nc.vector.tensor_scalar(
                        out=h_all[:, t, : N // 2],
                        in0=xts[t][:, : N // 2],
                        scalar1=scale_c,
                        scalar2=bias_c,
                        op0=ALU.mult,
                        op1=ALU.add,
                    )
                    nc.scalar.activation(
                        out=h_all[:, t, N // 2 :],
                        in_=xts[t][:, N // 2 :],
                        func=AF.Identity,
                        bias=bias_c,
                        scale=scale_c,
                    )
            # xp closed: x space freed

            with (
                tc.tile_pool(name="kqv", bufs=1) as kqv,
                tc.tile_pool(name="etp", bufs=1) as etp,
            ):
                k_all = kqv.tile([P, CT, N], fp8)
                q_all = kqv.tile([P, CT, NQ], fp8)
                v_all = kqv.tile([P, NKB, C], fp8)
                ao_all = kqv.tile([P, CT, 512], fp8)
                # prefetch the fp32 residual (+ output bias) while the sync
                # queue is idle, so epilogues never wait on residual DMA
                xrp = kqv.tile([P, CT, QCH, 512], f32)
                for co in range(CT):
                    nc.sync.dma_start(
                        xrp[:, co, :, :],
                        xr_t[co].rearrange("p (qc n) -> p qc n", n=512),
                    )
                    nc.vector.tensor_scalar_add(
                        out=xrp[:, co, :, :],
                        in0=xrp[:, co, :, :],
                        scalar1=cols["boc"][:, co : co + 1],
                    )
                et2 = [
                    etp.tile([P, 2, 512], fp8, tag=f"et{jj}", name=f"et{jj}")
                    for jj in range(NKB // 2)
                ]

                # ---- k [C, N], q [C, NQ] (DoubleRow over c-pairs) ----
                for t in range(CT):
                    for nb in range(N // 512):
                        ps = ps_s.tile([P, 512], f32, tag="s")
                        for i2 in range(2):
                            nc.tensor.matmul(
                                ps,
                                lhsT=w3["wk3"][:, 2 * i2 : 2 * i2 + 2, t * P : (t + 1) * P],
                                rhs=h_all[:, 2 * i2 : 2 * i2 + 2, nb * 512 : (nb + 1) * 512],
                                start=(i2 == 0),
                                stop=(i2 == 1),
                                perf_mode=DR,
                            )
                        nc.scalar.activation(
                            out=k_all[:, t, nb * 512 : (nb + 1) * 512],
                            in_=ps,
                            func=AF.Identity,
                            bias=cols["bkc"][:, t : t + 1],
                        )
                for t in range(CT):
                    for nb in range(NQ // 512):
                        ps = ps_s.tile([P, 512], f32, tag="s")
                        for i2 in range(2):
                            nc.tensor.matmul(
                                ps,
                                lhsT=w3["wq3"][:, 2 * i2 : 2 * i2 + 2, t * P : (t + 1) * P],
                                rhs=h_all[:, 2 * i2 : 2 * i2 + 2, nb * 512 : (nb + 1) * 512],
                                start=(i2 == 0),
                                stop=(i2 == 1),
                                perf_mode=DR,
                            )
                        # q eviction on DVE: keeps the ACT queue clear so the
                        # first chunk's exp stream starts without backlog
                        nc.vector.tensor_scalar_add(
                            out=q_all[:, t, nb * 512 : (nb + 1) * 512],
                            in0=ps,
                            scalar1=cols["bqc"][:, t : t + 1],
                        )

                # ---- vT [N, C] ----
                for nb in range(NKB):
                    ps = ps_o.tile([P, C], f32, tag="o")
                    for i2 in range(2):
                        nc.tensor.matmul(
                            ps,
                            lhsT=h_all[:, 2 * i2 : 2 * i2 + 2, nb * P : (nb + 1) * P],
                            rhs=w3["wv3"][:, 2 * i2 : 2 * i2 + 2, :],
                            start=(i2 == 0),
                            stop=(i2 == 1),
                            perf_mode=DR,
                        )
                    nc.vector.tensor_add(out=v_all[:, nb, :], in0=ps, in1=bv_bcast)

                # ---- attention ----
                # Two PE-dense phases per q-chunk with chunk-resident exp
                # tiles: (1) all scores + Exp evictions, (2) all PV + denom
                # matmuls. Chunks are software-pipelined: the next chunk's
                # scores phase is emitted before this chunk's epilogue.
                def scores_phase(qc):
                    qs = qc * 512
                    for j in range(NKB):
                        pss = ps_s.tile([P, 512], f32, tag="s", name="pss")
                        for i2 in range(2):
                            nc.tensor.matmul(
                                pss,
                                lhsT=k_all[:, 2 * i2 : 2 * i2 + 2, j * P : (j + 1) * P],
                                rhs=q_all[:, 2 * i2 : 2 * i2 + 2, qs : qs + 512],
                                start=(i2 == 0),
                                stop=(i2 == 1),
                                perf_mode=DR,
                            )
                        nc.scalar.activation(
                            out=et2[j // 2][:, j % 2, :],
                            in_=pss,
                            func=AF.Exp,
                            scale=SCALE,
                            bias=ebias,
                        )

                def pv_phase():
                    pso = [
                        ps_o.tile([P, 512], f32, tag="o", name="pso")
                        for _ in range(CT)
                    ]
                    psd = ps_d.tile([P, 512], f32, tag="d", name="psd")
                    for jj in range(NKB // 2):
                        for co in range(CT):
                            nc.tensor.matmul(
                                pso[co],
                                lhsT=v_all[:, 2 * jj : 2 * jj + 2, co * P : (co + 1) * P],
                                rhs=et2[jj],
                                start=(jj == 0),
                                stop=(jj == NKB // 2 - 1),
                                perf_mode=DR,
                            )
                        nc.tensor.matmul(
                            psd,
                            lhsT=ones3,
                            rhs=et2[jj],
                            start=(jj == 0),
                            stop=(jj == NKB // 2 - 1),
                            perf_mode=DR,
                        )
                    return pso, psd

                def evict_ao(pso, last):
                    # frees the 4 PV banks first thing so the next chunk's PV
                    # never queues behind the rest of the epilogue DVE chain;
                    # the last chunk uses ACT (its exp stream is finished) to
                    # keep the tail off the DVE queue
                    for ci in range(CT):
                        if last:
                            nc.scalar.mul(ao_all[:, ci, :], pso[ci], AO_SCALE)
                        else:
                            nc.vector.tensor_scalar_mul(
                                ao_all[:, ci, :], pso[ci], AO_SCALE
                            )

                def epilogue(qc, psd):
                    qs = qc * 512
                    rdb = work.tile([P, 512], f32, tag="rdb", bufs=2)
                    nc.vector.reciprocal(rdb, psd)
                    for co in range(CT):
                        psp = ps_o.tile([P, 512], f32, tag="o", name="psp")
                        for i2 in range(2):
                            nc.tensor.matmul(
                                psp,
                                lhsT=w3["wo3"][:, 2 * i2 : 2 * i2 + 2, co * P : (co + 1) * P],
                                rhs=ao_all[:, 2 * i2 : 2 * i2 + 2, :],
                                start=(i2 == 0),
                                stop=(i2 == 1),
                                perf_mode=DR,
                            )
                        osb = work.tile([P, 512], f32, tag="osb", bufs=3)
                        nc.vector.tensor_mul(osb, psp, rdb)
                        nc.vector.tensor_add(out=osb, in0=osb, in1=xrp[:, co, qc, :])
                        nc.sync.dma_start(out_t[co][:, qs : qs + 512], osb)

                scores_phase(0)
                for qc in range(QCH):
                    pso, psd = pv_phase()
                    evict_ao(pso, last=(qc == QCH - 1))
                    if qc + 1 < QCH:
                        scores_phase(qc + 1)
                    epilogue(qc, psd)

    nc.compile()
    return nc


def _get_nc():
    if "nc" not in _cache:
        _cache["nc"] = _build()
    return _cache["nc"]


def _prep_common(inputs):
    fp8 = ml_dtypes.float8_e4m3

    def colize(v):
        v = np.asarray(v, np.float32).reshape(CT, P)
        return np.ascontiguousarray(v.T)

    def w3(w):
        # [Cout, Cin] -> [p, cin_tile, cout] fp8 (DoubleRow stationary layout)
        t = np.asarray(w, np.float32).T.reshape(CT, P, C).transpose(1, 0, 2)
        return np.ascontiguousarray(t.astype(fp8))

    gmat = (np.arange(P)[:, None] // 16 == np.arange(8)[None, :]).astype(np.float32)

    return {
        "wq3": w3(inputs["wq"]),
        "wk3": w3(inputs["wk"]),
        "wv3": w3(inputs["wv"]),
        "wo3": w3(inputs["wo"]),
        "bqc": colize(inputs["bq"]),
        "bkc": colize(inputs["bk"]),
        "boc": colize(inputs["bo"]),
        "gnwc": colize(inputs["gn_w"]),
        "gnbc": colize(inputs["gn_b"]),
        "bvb": np.ascontiguousarray(
            np.tile(np.asarray(inputs["bv"], np.float32)[None, :], (P, 1))
        ),
        "gmat": gmat,
        "gtmat": np.ascontiguousarray(gmat.T),
    }


def make_in_maps(inputs):
    fp8 = ml_dtypes.float8_e4m3
    x = np.ascontiguousarray(np.asarray(inputs["hidden_states"], dtype=np.float32))
    B = x.shape[0]
    xs = x.reshape(B, C, N)
    common = _prep_common(inputs)
    in_maps = []
    for core in range(8):
        s, half = core // 2, core % 2
        xc = xs[s] if half == 0 else np.ascontiguousarray(np.roll(xs[s], -NQ, axis=1))
        in_maps.append(
            {
                "xb": np.ascontiguousarray(xc.astype(fp8)),
                "xr": np.ascontiguousarray(xc[:, :NQ]),
                **common,
            }
        )
    return in_maps


def kernel(**inputs):
    from concourse.bass_utils import run_bass_kernel_spmd

    nc = _get_nc()
    in_maps = make_in_maps(inputs)
    res = run_bass_kernel_spmd(nc, in_maps, list(range(8)))

    B = np.asarray(inputs["hidden_states"]).shape[0]
    out = np.empty((B, C, N), np.float32)
    for core in range(8):
        s, half = core // 2, core % 2
        out[s][:, half * NQ : (half + 1) * NQ] = res.results[core]["out"]
    return out.reshape(B, C, 64, 64)


# revision 39
# speedup vs baseline: 1.0067x; 1.0067x over previous
"""AttnBlock (GroupNorm + 4096-token single-head attention + residual) on 8 trn2 cores.

Sharding: 2 cores per batch sample. Each core computes GroupNorm + K/V for the
full sample (duplicated within the pair) and attention for half the queries
(2048 of 4096). A single SPMD program serves both halves: the host rotates the
sample's spatial columns so each core's query half always sits at columns
0..2047 (attention is permutation-invariant over keys; GroupNorm stats are
permutation-invariant over spatial positions).

All heavy matmuls run in fp8(e4m3) with perf_mode=DoubleRow (K=256 per pass,
2x PE throughput vs bf16; the DR matmul rate is moving-operand-stream-bound at
~216ns per [128,2,512]x[128,2,512] pass, LDWEIGHTS fully hidden). The softmax
exp is computed as exp(s*scale - 2.5) so the fp8 range (TRN e4m3 max = 240) is
never exceeded; the PV accumulators are scaled by 1/16 on both the numerator
(ao eviction scale) and denominator (ones-matrix value) so the fp8 ao tiles
stay in range and the factors cancel exactly at the final division. The
residual path stays exact fp32 (separate DMA of the fp32 input columns).

GroupNorm: each 16-channel group lives inside one 128-channel partition tile,
so stats are per-tile independent. x arrives pre-cast to fp8 (2x DVE/ACT stats
rate, half the DMA bytes; quantization noise is far below the fp8 h noise).
Per tile: DVE sum + ACT square-accumulate, then one tiny PE matmul against a
0/1 group-indicator (junk DoubleRow matmuls on the landed tile keep the PE HAM
warm through the stats pipeline). The scalar (mean,var->scale,bias) chain is
batched across all four tiles into a single DVE/ACT pass so its cross-engine
latency is paid once, then one broadcast matmul restores per-channel columns.
x owns the sync DMA queue; weights/consts ride the gpsimd (SWDGE) queue so
their descriptor pushes don't delay the ACT stats stream. Hard-won scheduling
rules, all measured: do NOT split x across two DMA queues or use half-tile
descriptors (globally slows every engine ~1.2x on this part); do NOT put
mid-chunk epilogue work on the ACT queue (FIFO behind the next chunk's exp
stream); do NOT use GpSimd tensor ops for the epilogue (slow).
"""

import sys

for _p in ("/opt/trn_rl_repo", "/root/.axon_site/_ro/trn_rl_repo"):
    if _p not in sys.path:
        sys.path.append(_p)

import ml_dtypes
import numpy as np

C = 512
N = 4096
NQ = 2048
P = 128
CT = C // P  # 4 c-tiles
NKB = N // P  # 32 nk blocks
QCH = NQ // 512  # 4 q chunks of 512
EPS = 1e-5
SCALE = float(C) ** -0.5
EXP_BIAS = -2.5  # exp(s*SCALE - 2.5): keeps fp8 et <= ~25; cancels in division
AO_SCALE = 1.0 / 16.0  # unnormalized PV sums reach ~550; scale into fp8 range

_cache = {}


def _build():
    import concourse.bacc as bacc
    import concourse.bass as bass
    import concourse.mybir as mybir
    import concourse.tile as tile

    f32 = mybir.dt.float32
    fp8 = mybir.dt.float8e4
    bf16 = mybir.dt.bfloat16
    AF = mybir.ActivationFunctionType
    ALU = mybir.AluOpType
    AX = mybir.AxisListType
    DR = mybir.MatmulPerfMode.DoubleRow

    nc = bacc.Bacc("TRN2", target_bir_lowering=False, debug=False, num_devices=8)

    xb_d = nc.dram_tensor("xb", [C, N], fp8, kind="ExternalInput")
    xr_d = nc.dram_tensor("xr", [C, NQ], f32, kind="ExternalInput")
    w3_d = {
        nm: nc.dram_tensor(nm, [P, CT, C], fp8, kind="ExternalInput")
        for nm in ("wq3", "wk3", "wv3", "wo3")
    }
    col_d = {
        nm: nc.dram_tensor(nm, [P, CT], f32, kind="ExternalInput")
        for nm in ("bqc", "bkc", "boc", "gnwc", "gnbc")
    }
    bvb_d = nc.dram_tensor("bvb", [P, C], f32, kind="ExternalInput")
    g_d = nc.dram_tensor("gmat", [P, 8], f32, kind="ExternalInput")
    gt_d = nc.dram_tensor("gtmat", [8, P], f32, kind="ExternalInput")
    out_d = nc.dram_tensor("out", [C, NQ], f32, kind="ExternalOutput")

    xb_t = xb_d.ap().rearrange("(t p) n -> t p n", p=P)
    xr_t = xr_d.ap().rearrange("(t p) n -> t p n", p=P)
    out_t = out_d.ap().rearrange("(t p) n -> t p n", p=P)

    with tile.TileContext(nc) as tc:
        with (
            tc.tile_pool(name="const", bufs=1) as const,
            tc.tile_pool(name="work", bufs=3) as work,
            tc.tile_pool(name="wtp", bufs=1) as wtp,
            tc.tile_pool(name="hp", bufs=1) as hp,
            tc.tile_pool(name="ps_s", bufs=2, space="PSUM") as ps_s,
            tc.tile_pool(name="ps_o", bufs=4, space="PSUM") as ps_o,
            tc.tile_pool(name="ps_d", bufs=2, space="PSUM") as ps_d,
        ):
            # ---- constants (vector DMA queue; x owns sync, epilogue reuses it) ----
            ones3 = const.tile([P, 2, P], fp8)
            nc.vector.memset(ones3, AO_SCALE)
            eps8 = const.tile([8, 1], f32)
            nc.vector.memset(eps8, EPS)
            ebias = const.tile([P, 1], f32)
            nc.vector.memset(ebias, EXP_BIAS)
            gmat = const.tile([P, 8], f32)
            nc.gpsimd.dma_start(gmat, g_d.ap())
            gtmat = const.tile([8, P], f32)
            nc.gpsimd.dma_start(gtmat, gt_d.ap())

            cols = {}
            for nm in ("bqc", "bkc", "boc", "gnwc", "gnbc"):
                cols[nm] = const.tile([P, CT], f32, tag=f"c_{nm}", name=f"c_{nm}")
                nc.gpsimd.dma_start(cols[nm], col_d[nm].ap())
            bv_bcast = const.tile([P, C], f32)
            nc.gpsimd.dma_start(bv_bcast, bvb_d.ap())

            w3 = {}
            for nm in ("wk3", "wq3", "wv3", "wo3"):
                w3[nm] = wtp.tile([P, CT, C], fp8, tag=nm, name=nm)
                nc.gpsimd.dma_start(w3[nm], w3_d[nm].ap())

            h_all = hp.tile([P, CT, N], fp8)

            with tc.tile_pool(name="xp", bufs=1) as xp:
                xts = []
                psg_all = ps_d.tile([8, 2 * CT], f32, tag="d", name="psg_all")
                for t in range(CT):
                    xt = xp.tile([P, N], fp8, tag=f"x{t}", name=f"x{t}")
                    nc.sync.dma_start(xt, xb_t[t])
                    xts.append(xt)

                    # per-tile raw stats: DVE sum + ACT square-accumulate
                    # (free-axis reductions only exist on DVE/ACT; both are
                    # saturated here, so this pipeline is the preamble floor)
                    st2 = work.tile([P, 2], f32, tag="st2", bufs=2)
                    nc.vector.tensor_reduce(
                        out=st2[:, 0:1], in_=xt, axis=AX.X, op=ALU.add
                    )
                    junk = xp.tile([P, N], bf16, tag="junk", bufs=2)
                    nc.scalar.activation(
                        out=junk, in_=xt, func=AF.Square, accum_out=st2[:, 1:2]
                    )
                    nc.tensor.matmul(
                        psg_all[:, 2 * t : 2 * t + 2],
                        lhsT=gmat,
                        rhs=st2,
                        start=True,
                        stop=True,
                    )
                    # HAM warm-up: junk matmuls on the already-landed x tile
                    # fill the PE gaps of the stats pipeline so the k-phase
                    # starts at 2.4GHz instead of cold 1.2GHz
                    for _w in range(8):
                        jps = ps_s.tile([P, 512], f32, tag="s", name="jps")
                        nc.tensor.matmul(
                            jps,
                            lhsT=ones3,
                            rhs=xt[:, :1024].rearrange("p (k n) -> p k n", k=2),
                            start=True,
                            stop=True,
                            perf_mode=DR,
                        )

                # ---- batched (mean,var)->(scale,bias) chain for all tiles ----
                mq8 = work.tile([8, 4, 2], f32)  # [g, t, (mean, E[x^2])]
                nc.scalar.mul(
                    mq8, psg_all.rearrange("g (t c) -> g t c", c=2), 1.0 / (16.0 * 4096.0)
                )
                var4 = work.tile([8, 4], f32)
                nc.vector.tensor_mul(var4, mq8[:, :, 0], mq8[:, :, 0])
                nc.vector.tensor_sub(var4, mq8[:, :, 1], var4)
                nc.scalar.activation(out=var4, in_=var4, func=AF.Sqrt, bias=eps8)
                rstd4 = work.tile([8, 4], f32)
                nc.vector.reciprocal(rstd4, var4)
                mr8 = work.tile([8, 4, 2], f32)  # [g, t, (mean, rstd)]
                nc.vector.tensor_copy(mr8[:, :, 0], mq8[:, :, 0])
                nc.vector.tensor_copy(mr8[:, :, 1], rstd4)
                psc8 = ps_d.tile([P, 8], f32, tag="d", name="psc8")
                nc.tensor.matmul(psc8, lhsT=gtmat, rhs=mr8, start=True, stop=True)
                # per-channel scale/bias columns + fp8 h apply (DVE/ACT split)
                for t in range(CT):
                    scale_c = work.tile([P, 1], f32, tag="scale_c", bufs=2)
                    nc.vector.tensor_mul(
                        scale_c, psc8[:, 2 * t + 1 : 2 * t + 2], cols["gnwc"][:, t : t + 1]
                    )
                    bias_c = work.tile([P, 1], f32, tag="bias_c", bufs=2)
                    nc.vector.tensor_mul(bias_c, psc8[:, 2 * t : 2 * t + 1], scale_c)
                    nc.vector.tensor_sub(bias_c, cols["gnbc"][:, t : t + 1], bias_c)
                    nc.vector.tensor_scalar(
                        out=h_all[:, t, : N // 2],
                        in0=xts[t][:, : N // 2],
                        scalar1=scale_c,
                        scalar2=bias_c,
                        op0=ALU.mult,
                        op1=ALU.add,
                    )
                    nc.scalar.activation(
                        out=h_all[:, t, N // 2 :],
                        in_=xts[t][:, N // 2 :],
                        func=AF.Identity,
                        bias=bias_c,
                        scale=scale_c,
                    )
            # xp closed: x space freed

            with (
                tc.tile_pool(name="kqv", bufs=1) as kqv,
                tc.tile_pool(name="etp", bufs=1) as etp,
            ):
                k_all = kqv.tile([P, CT, N], fp8)
                q_all = kqv.tile([P, CT, NQ], fp8)
                v_all = kqv.tile([P, NKB, C], fp8)
                ao_all = kqv.tile([P, CT, 512], fp8)
                # prefetch the fp32 residual (+ output bias) while the sync
                # queue is idle, so epilogues never wait on residual DMA
                xrp = kqv.tile([P, CT, QCH, 512], f32)
                for co in range(CT):
                    nc.sync.dma_start(
                        xrp[:, co, :, :],
                        xr_t[co].rearrange("p (qc n) -> p qc n", n=512),
                    )
                    nc.vector.tensor_scalar_add(
                        out=xrp[:, co, :, :],
                        in0=xrp[:, co, :, :],
                        scalar1=cols["boc"][:, co : co + 1],
                    )
                et2 = [
                    etp.tile([P, 2, 512], fp8, tag=f"et{jj}", name=f"et{jj}")
                    for jj in range(NKB // 2)
                ]

                # ---- k [C, N], q [C, NQ] (DoubleRow over c-pairs) ----
                for t in range(CT):
                    for nb in range(N // 512):
                        ps = ps_s.tile([P, 512], f32, tag="s")
                        for i2 in range(2):
                            nc.tensor.matmul(
                                ps,
                                lhsT=w3["wk3"][:, 2 * i2 : 2 * i2 + 2, t * P : (t + 1) * P],
                                rhs=h_all[:, 2 * i2 : 2 * i2 + 2, nb * 512 : (nb + 1) * 512],
                                start=(i2 == 0),
                                stop=(i2 == 1),
                                perf_mode=DR,
                            )
                        nc.scalar.activation(
                            out=k_all[:, t, nb * 512 : (nb + 1) * 512],
                            in_=ps,
                            func=AF.Identity,
                            bias=cols["bkc"][:, t : t + 1],
                        )
                for t in range(CT):
                    for nb in range(NQ // 512):
                        ps = ps_s.tile([P, 512], f32, tag="s")
                        for i2 in range(2):
                            nc.tensor.matmul(
                                ps,
                                lhsT=w3["wq3"][:, 2 * i2 : 2 * i2 + 2, t * P : (t + 1) * P],
                                rhs=h_all[:, 2 * i2 : 2 * i2 + 2, nb * 512 : (nb + 1) * 512],
                                start=(i2 == 0),
                                stop=(i2 == 1),
                                perf_mode=DR,
                            )
                        nc.scalar.activation(
                            out=q_all[:, t, nb * 512 : (nb + 1) * 512],
                            in_=ps,
                            func=AF.Identity,
                            bias=cols["bqc"][:, t : t + 1],
                        )

                # ---- vT [N, C] ----
                for nb in range(NKB):
                    ps = ps_o.tile([P, C], f32, tag="o")
                    for i2 in range(2):
                        nc.tensor.matmul(
                            ps,
                            lhsT=h_all[:, 2 * i2 : 2 * i2 + 2, nb * P : (nb + 1) * P],
                            rhs=w3["wv3"][:, 2 * i2 : 2 * i2 + 2, :],
                            start=(i2 == 0),
                            stop=(i2 == 1),
                            perf_mode=DR,
                        )
                    nc.vector.tensor_add(out=v_all[:, nb, :], in0=ps, in1=bv_bcast)

                # ---- attention ----
                # Two PE-dense phases per q-chunk with chunk-resident exp
                # tiles: (1) all scores + Exp evictions, (2) all PV + denom
                # matmuls. Chunks are software-pipelined: the next chunk's
                # scores phase is emitted before this chunk's epilogue.
                def scores_phase(qc):
                    qs = qc * 512
                    for j in range(NKB):
                        pss = ps_s.tile([P, 512], f32, tag="s", name="pss")
                        for i2 in range(2):
                            nc.tensor.matmul(
                                pss,
                                lhsT=k_all[:, 2 * i2 : 2 * i2 + 2, j * P : (j + 1) * P],
                                rhs=q_all[:, 2 * i2 : 2 * i2 + 2, qs : qs + 512],
                                start=(i2 == 0),
                                stop=(i2 == 1),
                                perf_mode=DR,
                            )
                        nc.scalar.activation(
                            out=et2[j // 2][:, j % 2, :],
                            in_=pss,
                            func=AF.Exp,
                            scale=SCALE,
                            bias=ebias,
                        )

                def pv_phase():
                    pso = [
                        ps_o.tile([P, 512], f32, tag="o", name="pso")
                        for _ in range(CT)
                    ]
                    psd = ps_d.tile([P, 512], f32, tag="d", name="psd")
                    for jj in range(NKB // 2):
                        for co in range(CT):
                            nc.tensor.matmul(
                                pso[co],
                                lhsT=v_all[:, 2 * jj : 2 * jj + 2, co * P : (co + 1) * P],
                                rhs=et2[jj],
                                start=(jj == 0),
                                stop=(jj == NKB // 2 - 1),
                                perf_mode=DR,
                            )
                        nc.tensor.matmul(
                            psd,
                            lhsT=ones3,
                            rhs=et2[jj],
                            start=(jj == 0),
                            stop=(jj == NKB // 2 - 1),
                            perf_mode=DR,
                        )
                    return pso, psd

                def evict_ao(pso, last):
                    # frees the 4 PV banks first thing so the next chunk's PV
                    # never queues behind the rest of the epilogue DVE chain;
                    # the last chunk uses ACT (its exp stream is finished) to
                    # keep the tail off the DVE queue
                    for ci in range(CT):
                        if last:
                            nc.scalar.mul(ao_all[:, ci, :], pso[ci], AO_SCALE)
                        else:
                            nc.vector.tensor_scalar_mul(
                                ao_all[:, ci, :], pso[ci], AO_SCALE
                            )

                def epilogue(qc, psd):
                    qs = qc * 512
                    rdb = work.tile([P, 512], f32, tag="rdb", bufs=2)
                    nc.vector.reciprocal(rdb, psd)
                    for co in range(CT):
                        psp = ps_o.tile([P, 512], f32, tag="o", name="psp")
                        for i2 in range(2):
                            nc.tensor.matmul(
                                psp,
                                lhsT=w3["wo3"][:, 2 * i2 : 2 * i2 + 2, co * P : (co + 1) * P],
                                rhs=ao_all[:, 2 * i2 : 2 * i2 + 2, :],
                                start=(i2 == 0),
                                stop=(i2 == 1),
                                perf_mode=DR,
                            )
                        osb = work.tile([P, 512], f32, tag="osb", bufs=3)
                        nc.vector.tensor_mul(osb, psp, rdb)
                        nc.vector.tensor_add(out=osb, in0=osb, in1=xrp[:, co, qc, :])
                        nc.sync.dma_start(out_t[co][:, qs : qs + 512], osb)

                scores_phase(0)
                for qc in range(QCH):
                    pso, psd = pv_phase()
                    evict_ao(pso, last=(qc == QCH - 1))
                    if qc + 1 < QCH:
                        scores_phase(qc + 1)
                    epilogue(qc, psd)

    nc.compile()
    return nc


def _get_nc():
    if "nc" not in _cache:
        _cache["nc"] = _build()
    return _cache["nc"]


def _prep_common(inputs):
    fp8 = ml_dtypes.float8_e4m3

    def colize(v):
        v = np.asarray(v, np.float32).reshape(CT, P)
        return np.ascontiguousarray(v.T)

    def w3(w):
        # [Cout, Cin] -> [p, cin_tile, cout] fp8 (DoubleRow stationary layout)
        t = np.asarray(w, np.float32).T.reshape(CT, P, C).transpose(1, 0, 2)
        return np.ascontiguousarray(t.astype(fp8))

    gmat = (np.arange(P)[:, None] // 16 == np.arange(8)[None, :]).astype(np.float32)

    return {
        "wq3": w3(inputs["wq"]),
        "wk3": w3(inputs["wk"]),
        "wv3": w3(inputs["wv"]),
        "wo3": w3(inputs["wo"]),
        "bqc": colize(inputs["bq"]),
        "bkc": colize(inputs["bk"]),
        "boc": colize(inputs["bo"]),
        "gnwc": colize(inputs["gn_w"]),
        "gnbc": colize(inputs["gn_b"]),
        "bvb": np.ascontiguousarray(
            np.tile(np.asarray(inputs["bv"], np.float32)[None, :], (P, 1))
        ),
        "gmat": gmat,
        "gtmat": np.ascontiguousarray(gmat.T),
    }


def make_in_maps(inputs):
    fp8 = ml_dtypes.float8_e4m3
    x = np.ascontiguousarray(np.asarray(inputs["hidden_states"], dtype=np.float32))
    B = x.shape[0]
    xs = x.reshape(B, C, N)
    common = _prep_common(inputs)
    in_maps = []
    for core in range(8):
        s, half = core // 2, core % 2
        xc = xs[s] if half == 0 else np.ascontiguousarray(np.roll(xs[s], -NQ, axis=1))
        in_maps.append(
            {
                "xb": np.ascontiguousarray(xc.astype(fp8)),
                "xr": np.ascontiguousarray(xc[:, :NQ]),
                **common,
            }
        )
    return in_maps


def kernel(**inputs):
    from concourse.bass_utils import run_bass_kernel_spmd

    nc = _get_nc()
    in_maps = make_in_maps(inputs)
    res = run_bass_kernel_spmd(nc, in_maps, list(range(8)))

    B = np.asarray(inputs["hidden_states"]).shape[0]
    out = np.empty((B, C, N), np.float32)
    for core in range(8):
        s, half = core // 2, core % 2
        out[s][:, half * NQ : (half + 1) * NQ] = res.results[core]["out"]
    return out.reshape(B, C, 64, 64)
